# revision 33
# baseline (speedup 1.0000x reference)
"""Trainium2 Bass kernel for nn_EnhancedFlowLayer (topk_masking) — v2.

8 cores. Tokens on partitions (2 groups of 128); flow (i,j)-space sharded by i
across cores (64 i-rows -> 32768 elems/token/core). Patterns live resident in
SBUF in a 4-way row-tiled layout; F is rematerialized on the PE per phase with
4 concurrent K=16 fp32 matmuls (tile_position row groups). Exact per-token
rank-kk threshold via: bf16 |F| + sampled Newton + 5-rung count ladder (counts
split across scalar[Sign-accum] and vector engines; one all-reduce) + max8
band extraction (top-8 per 512-chunk) + one all-gather + replicated exact fp32
bisection (9 full + 17 top-8 iters). Final pass recomputes F, applies mask,
does the masked matvec, one all-gather of flow_out slices, then a replicated
LN2 + memory-MLP + FFN tail.
"""

import os
from contextlib import ExitStack

import numpy as np

B, S, D, P = 1, 256, 512, 16
MAX_SEQ = 4096
NCORES = 8
ISLICE = D // NCORES          # 64 i-rows per core
FREE = ISLICE * D             # 32768 ij elements per token per core
NG = 2                        # token groups of 128
DD = D * D
HF = FREE // 2                # 16384 (one extraction half)
EXCH = 256                    # band-extraction chunk width (top-8 per chunk)
NCAND = (FREE // EXCH) * 8    # 1024 candidate slots per token per core
NL = 5                        # ladder rungs
N_BISECT = 9
N_BISECT2 = 14
QW = 4096                     # count-op quarter width (8 per rung)
NQ = FREE // QW               # 8
NEWT_W = 2048                 # Newton sample width (contiguous)
SCAL_UNITS = (0, 2, 4, 6, 8)  # (g,rung) count units on ACT (parity split)

DEBUG = os.environ.get("KERNEL_DEBUG", "0") == "1"
STAGE = int(os.environ.get("KERNEL_STAGE", "3"))
SIM_COMPAT = os.environ.get("KERNEL_SIM_COMPAT", "0") == "1"


def _host_constants():
    pos = np.arange(S, dtype=np.float64)
    inv = 1.0 / (10000.0 ** (np.arange(0, D, 2, dtype=np.float64) / D))
    ang = pos[:, None] * inv[None, :]
    sin = np.repeat(np.sin(ang), 2, axis=-1).astype(np.float32)
    cos = np.repeat(np.cos(ang), 2, axis=-1).astype(np.float32)
    # half-normal tail quantile z(q): P(|N(0,1)| >= z) = q, cubic in ln q
    qpoly = np.array([-0.0036756, -0.06789169, -0.73664117, 0.26370117], np.float32)
    return sin, cos, qpoly


def build_kernel():
    import concourse.bass as bass
    import concourse.mybir as mybir
    from concourse import bacc, masks
    from concourse.tile import TileContext

    dt = mybir.dt
    Alu = mybir.AluOpType
    Act = mybir.ActivationFunctionType
    AxX = mybir.AxisListType.X
    f32, bf16 = dt.float32, dt.bfloat16

    nc = bacc.Bacc("TRN2", num_devices=NCORES)

    dp = nc.declare_dram_parameter
    x_in = dp("x", [S, D], f32, isOutput=False)
    pat_sl = dp("pat_sl", [P, FREE], f32, isOutput=False)
    sel_w1 = dp("sel_w1", [2 * D, 2 * P], f32, isOutput=False)
    sel_b1 = dp("sel_b1", [1, 2 * P], f32, isOutput=False)
    sel_w2 = dp("sel_w2", [2 * P, P], f32, isOutput=False)
    sel_b2 = dp("sel_b2", [1, P], f32, isOutput=False)
    win_w1 = dp("win_w1", [D, 64], f32, isOutput=False)
    win_b1 = dp("win_b1", [1, 64], f32, isOutput=False)
    win_w2 = dp("win_w2", [64, 1], f32, isOutput=False)
    win_b2 = dp("win_b2", [1, 1], f32, isOutput=False)
    int_w1 = dp("int_w1", [2 * D, 64], f32, isOutput=False)
    int_b1 = dp("int_b1", [1, 64], f32, isOutput=False)
    int_w2 = dp("int_w2", [64, 1], f32, isOutput=False)
    int_b2 = dp("int_b2", [1, 1], f32, isOutput=False)
    mem_w1 = dp("mem_w1", [2 * D, D], f32, isOutput=False)
    mem_b1 = dp("mem_b1", [1, D], f32, isOutput=False)
    mem_w2 = dp("mem_w2", [D, D], f32, isOutput=False)
    mem_b2 = dp("mem_b2", [1, D], f32, isOutput=False)
    memory_bank = dp("memory_bank", [512, D], f32, isOutput=False)
    up_w = dp("up_w", [D, 8 * D], f32, isOutput=False)
    up_b = dp("up_b", [1, 8 * D], f32, isOutput=False)
    down_w = dp("down_w", [4 * D, D], f32, isOutput=False)
    down_b = dp("down_b", [1, D], f32, isOutput=False)
    n1_g = dp("n1_g", [1, D], f32, isOutput=False)
    n1_b = dp("n1_b", [1, D], f32, isOutput=False)
    n2_g = dp("n2_g", [1, D], f32, isOutput=False)
    n2_b = dp("n2_b", [1, D], f32, isOutput=False)
    rope_sin = dp("rope_sin", [S, D], f32, isOutput=False)
    rope_cos = dp("rope_cos", [S, D], f32, isOutput=False)
    qpoly = dp("qpoly", [1, 4], f32, isOutput=False)
    # host-converted bf16 copies of the tail weights (boundary-flip noise
    # dominates the error budget; bf16 tail matmuls are free — verified)
    mem_w1b = dp("mem_w1b", [D, D], dt.bfloat16, isOutput=False)
    mem_w1cb = dp("mem_w1cb", [D, D], dt.bfloat16, isOutput=False)
    mem_w2b = dp("mem_w2b", [D, D], dt.bfloat16, isOutput=False)
    up_wb = dp("up_wb", [D, 8 * D], dt.bfloat16, isOutput=False)
    down_wb = dp("down_wb", [4 * D, D], dt.bfloat16, isOutput=False)
    out_dram = dp("out", [S, D], f32, isOutput=True)

    dbg = {}
    if DEBUG:
        for name, shape in [
            ("dbg_xn", [S, D]), ("dbg_xr", [S, D]), ("dbg_pw", [S, P]),
            ("dbg_inten", [S, 1]), ("dbg_scal", [1, 8]), ("dbg_t0", [S, 1]),
            ("dbg_cnt", [S, 8]), ("dbg_beta", [S, 4]), ("dbg_th", [S, 2]),
            ("dbg_fo", [S, D]), ("dbg_cand", [S, NCAND]),
        ]:
            dbg[name] = dp(name, shape, f32, isOutput=True)

    RG = [list(range(NCORES))]

    with ExitStack() as ctx:
        tc = ctx.enter_context(TileContext(nc))
        # persistent small state (lives for whole kernel)
        pw_ = ctx.enter_context(tc.tile_pool(name="persist", bufs=1))
        # PSUM: 3 x [128,1024] double-bank matmul tiles + 2 banks for misc
        pool_mm = ctx.enter_context(tc.tile_pool(name="psumMM", bufs=3,
                                                 space="PSUM"))
        pool_ps = ctx.enter_context(tc.tile_pool(name="psumT", bufs=2,
                                                 space="PSUM"))
        pool_dram = ctx.enter_context(tc.tile_pool(name="dramst", bufs=1,
                                                   space="DRAM"))

        def dma(dst, src):
            nc.sync.dma_start(out=dst, in_=src)

        def bcast_row(pool, src_dram_row, width, name, dtype=f32, tag=None,
                      col0=0):
            t = pool.tile([128, width], dtype, name=name, tag=tag)
            dma(t[:], src_dram_row[:, col0:col0 + width].to_broadcast(
                [128, width]))
            return t

        identity = pw_.tile([128, 128], f32, name="identity")
        masks.make_identity(nc, identity[:])
        bc_n = [0]

        def pbcast(pool, dst_ap, src_ap, width, name):
            """broadcast [1,width] sbuf row to [128,width] via a DRAM bounce"""
            bc_n[0] += 1
            st = pool_dram.tile([1, width], f32, name=f"bc{bc_n[0]}_{name}")
            dma(st[:], src_ap)
            dma(dst_ap, st[:].to_broadcast([128, width]))

        def transpose_to(dst_ap, src_ap, name):
            p, f = src_ap.shape[0], src_ap.free_size()
            ps = pool_ps.tile([f, p], f32, name="Tps", tag="Tps",
                              padded_shape=[128, 128])
            nc.tensor.transpose(ps[:f, :p], src_ap, identity[:p, :p])
            nc.vector.tensor_copy(dst_ap, ps[:f, :p])

        ERF_FN = Act.Tanh if SIM_COMPAT else Act.Erf

        def gelu_(pool, ap, name):
            """in-place exact gelu: x * 0.5*(1+erf(x/sqrt(2)))"""
            e = pool.tile(list(ap.shape), f32, name=f"{name}_erf", tag="gelu_e")
            nc.scalar.activation(e[:], ap, ERF_FN, scale=float(1 / np.sqrt(2)))
            nc.vector.tensor_scalar(e[:], e[:], 1.0, 0.5, Alu.add, Alu.mult)
            nc.vector.tensor_tensor(ap, ap, e[:], Alu.mult)

        def silu_(pool, dst_ap, src_ap, name):
            sg = pool.tile(list(src_ap.shape), f32, name=f"{name}_sg",
                           tag="silu_s")
            nc.scalar.activation(sg[:], src_ap, Act.Sigmoid)
            nc.vector.tensor_tensor(dst_ap, src_ap, sg[:], Alu.mult)

        # ---------- persistent tiles ----------
        xg = [pw_.tile([128, D], f32, name=f"xg{g}") for g in range(NG)]
        xn = [pw_.tile([128, D], f32, name=f"xn{g}") for g in range(NG)]
        pwt4 = [pw_.tile([128, 128], f32, name=f"pwT4_{g}") for g in range(NG)]
        inten = [pw_.tile([128, 1], f32, name=f"inten{g}") for g in range(NG)]
        kk_b = pw_.tile([128, 1], f32, name="kk_b")
        zq_b = pw_.tile([128, 1], f32, name="zq_b")
        delta_b = pw_.tile([128, 1], f32, name="delta_b")
        invz2_b = pw_.tile([128, 1], f32, name="invz2_b")
        ones_sb = pw_.tile([128, 1], f32, name="ones_sb")
        nc.vector.memset(ones_sb[:], 1.0)
        beta = [(pw_.tile([128, 1], f32, name=f"b1t{g}"),
                 pw_.tile([128, 1], f32, name=f"b2t{g}")) for g in range(NG)]
        rprime = [pw_.tile([128, 1], f32, name=f"rp{g}") for g in range(NG)]
        th = [pw_.tile([128, 1], f32, name=f"th{g}") for g in range(NG)]
        blad_all = [pw_.tile([128, NL], f32, name=f"blad{g}")
                    for g in range(NG)]
        # patterns resident, 4-way row-tiled: partition 32q+p holds chunk
        # 4w+q of this core's i-slice at free offset w*512
        patall = pw_.tile([128, 16 * 512], f32, name="patall")
        pat4 = pat_sl[:].rearrange("p (w four v) -> p w four v", four=4, v=512)
        for q in range(4):
            dma(patall[32 * q:32 * q + P, :].rearrange(
                    "p (w v) -> p w v", v=512),
                pat4[:, :, q, :])

        for g in range(NG):
            dma(xg[g][:], x_in[g * 128:(g + 1) * 128, :])

        # =================== preamble (scoped pool) ===================
        with tc.tile_pool(name="preamble", bufs=1) as pp:
            sin_g, cos_g, xr = [], [], []
            for g in range(NG):
                t = pp.tile([128, D], f32, name=f"sin{g}")
                dma(t[:], rope_sin[g * 128:(g + 1) * 128, :])
                sin_g.append(t)
                t = pp.tile([128, D], f32, name=f"cos{g}")
                dma(t[:], rope_cos[g * 128:(g + 1) * 128, :])
                cos_g.append(t)
            n1g_b = bcast_row(pp, n1_g, D, "n1g_b")
            n1b_b = bcast_row(pp, n1_b, D, "n1b_b")

            for g in range(NG):
                mean = pp.tile([128, 1], f32, name=f"mean{g}")
                m2 = pp.tile([128, 1], f32, name=f"m2ln{g}")
                tmp = pp.tile([128, D], f32, name=f"lntmp{g}")
                nc.vector.tensor_reduce(mean[:], xg[g][:], AxX, Alu.add)
                nc.vector.tensor_scalar(mean[:], mean[:], 1.0 / D, None,
                                        Alu.mult)
                nc.vector.tensor_scalar(tmp[:], xg[g][:], mean[:], None,
                                        Alu.subtract)
                nc.vector.scalar_tensor_tensor(tmp[:], tmp[:], 1.0, tmp[:],
                                               Alu.mult, Alu.mult,
                                               accum_out=m2[:])
                nc.vector.tensor_scalar(m2[:], m2[:], 1.0 / D, 1e-5, Alu.mult,
                                        Alu.add)
                rstd = pp.tile([128, 1], f32, name=f"rstd{g}")
                nc.scalar.activation(rstd[:], m2[:], Act.Sqrt)
                nc.vector.reciprocal(rstd[:], rstd[:])
                nc.vector.tensor_scalar(xn[g][:], xg[g][:], mean[:], rstd[:],
                                        Alu.subtract, Alu.mult)
                nc.vector.scalar_tensor_tensor(xn[g][:], xn[g][:], 1.0,
                                               n1g_b[:], Alu.mult, Alu.mult)
                nc.vector.tensor_tensor(xn[g][:], xn[g][:], n1b_b[:], Alu.add)
                t_xr = pp.tile([128, D], f32, name=f"xr{g}")
                rot = pp.tile([128, D], f32, name=f"rot{g}")
                ev = lambda a: a.rearrange("p (a two) -> p a two", two=2)[:, :, 0]
                od = lambda a: a.rearrange("p (a two) -> p a two", two=2)[:, :, 1]
                nc.vector.tensor_scalar(ev(rot[:]), od(xn[g][:]), -1.0, None,
                                        Alu.mult)
                nc.vector.tensor_copy(od(rot[:]), ev(xn[g][:]))
                nc.vector.tensor_tensor(rot[:], rot[:], sin_g[g][:], Alu.mult)
                nc.vector.scalar_tensor_tensor(t_xr[:], xn[g][:], 1.0,
                                               cos_g[g][:], Alu.mult, Alu.mult)
                nc.vector.tensor_tensor(t_xr[:], t_xr[:], rot[:], Alu.add)
                xr.append(t_xr)

            # ctx = mean over tokens
            ctx_ps = pool_ps.tile([1, D], f32, name="ctx_ps", tag="Tps",
                                  padded_shape=[128, 512])
            for g in range(NG):
                nc.tensor.matmul(ctx_ps[:1, :], ones_sb[:], xr[g][:],
                                 start=(g == 0), stop=(g == NG - 1))
            ctx_row = pp.tile([1, D], f32, name="ctx_row")
            nc.vector.tensor_scalar(ctx_row[:], ctx_ps[:1, :], 1.0 / S, None,
                                    Alu.mult)

            xrT = pp.tile([128, 4 * S], f32, name="xrT")
            for g in range(NG):
                for kc in range(4):
                    transpose_to(xrT[:, kc * S + g * 128: kc * S + (g + 1) * 128],
                                 xr[g][:, kc * 128:(kc + 1) * 128], f"xrT{g}{kc}")
            ctxT = pp.tile([128, 4], f32, name="ctxT")
            for kc in range(4):
                transpose_to(ctxT[:, kc:kc + 1],
                             ctx_row[:, kc * 128:(kc + 1) * 128], f"ctxT{kc}")

            def mlp_head(w1, b1, w2, b2, h1_dim, h2_dim, name):
                w1a = pp.tile([128, 4 * h1_dim], f32, name=f"{name}_w1a")
                w1b = pp.tile([128, 4 * h1_dim], f32, name=f"{name}_w1b")
                for kc in range(4):
                    dma(w1a[:, kc * h1_dim:(kc + 1) * h1_dim],
                        w1[kc * 128:(kc + 1) * 128, :])
                    dma(w1b[:, kc * h1_dim:(kc + 1) * h1_dim],
                        w1[D + kc * 128: D + (kc + 1) * 128, :])
                b1_b = bcast_row(pp, b1, h1_dim, f"{name}_b1b")
                w2_sb = pp.tile([h1_dim, h2_dim], f32, name=f"{name}_w2sb")
                dma(w2_sb[:], w2[:])
                b2_b = bcast_row(pp, b2, h2_dim, f"{name}_b2b")
                v1_ps = pool_ps.tile([1, h1_dim], f32, name="v1ps", tag="Tps",
                                     padded_shape=[128, 128])
                for kc in range(4):
                    nc.tensor.matmul(v1_ps[:1, :], ctxT[:, kc:kc + 1],
                                     w1b[:, kc * h1_dim:(kc + 1) * h1_dim],
                                     start=(kc == 0), stop=(kc == 3))
                v1 = pp.tile([1, h1_dim], f32, name=f"{name}_v1")
                nc.vector.tensor_copy(v1[:], v1_ps[:1, :])
                v1_b = pp.tile([128, h1_dim], f32, name=f"{name}_v1b")
                pbcast(pp, v1_b[:], v1[:], h1_dim, f"{name}v1")
                outs = []
                for g in range(NG):
                    h1_ps = pool_ps.tile([128, h1_dim], f32, name="h1ps",
                                         tag="Tps", padded_shape=[128, 128])
                    for kc in range(4):
                        nc.tensor.matmul(
                            h1_ps[:],
                            xrT[:, kc * S + g * 128: kc * S + (g + 1) * 128],
                            w1a[:, kc * h1_dim:(kc + 1) * h1_dim],
                            start=(kc == 0), stop=(kc == 3))
                    h1 = pp.tile([128, h1_dim], f32, name=f"{name}_h1_{g}")
                    nc.vector.tensor_tensor(h1[:], h1_ps[:], v1_b[:], Alu.add)
                    nc.vector.tensor_tensor(h1[:], h1[:], b1_b[:], Alu.add)
                    gelu_(pp, h1[:], f"{name}g{g}")
                    h1T = pp.tile([h1_dim, 128], f32, name=f"{name}_h1T_{g}")
                    transpose_to(h1T[:], h1[:], f"{name}h1T{g}")
                    h2_ps = pool_ps.tile([128, h2_dim], f32, name="h2ps",
                                         tag="Tps", padded_shape=[128, 128])
                    nc.tensor.matmul(h2_ps[:], h1T[:], w2_sb[:], start=True,
                                     stop=True)
                    h2 = pp.tile([128, h2_dim], f32, name=f"{name}_h2_{g}")
                    nc.vector.tensor_tensor(h2[:], h2_ps[:], b2_b[:], Alu.add)
                    outs.append(h2)
                return outs

            sel_h2 = mlp_head(sel_w1, sel_b1, sel_w2, sel_b2, 2 * P, P, "sel")
            int_h2 = mlp_head(int_w1, int_b1, int_w2, int_b2, 64, 1, "intm")

            for g in range(NG):
                t_pw = pp.tile([128, P], f32, name=f"pwsm{g}")
                mx = pp.tile([128, 1], f32, name=f"selmx{g}")
                nc.vector.tensor_reduce(mx[:], sel_h2[g][:], AxX, Alu.max)
                nc.vector.tensor_scalar(sel_h2[g][:], sel_h2[g][:], mx[:],
                                        None, Alu.subtract)
                nc.scalar.activation(sel_h2[g][:], sel_h2[g][:], Act.Exp)
                sm = pp.tile([128, 1], f32, name=f"selsm{g}")
                nc.vector.tensor_reduce(sm[:], sel_h2[g][:], AxX, Alu.add)
                rs = pp.tile([128, 1], f32, name=f"selrs{g}")
                nc.vector.reciprocal(rs[:], sm[:])
                nc.vector.tensor_scalar(t_pw[:], sel_h2[g][:], rs[:], None,
                                        Alu.mult)
                nc.scalar.activation(inten[g][:], int_h2[g][:], Act.Sigmoid)
                # pw transposed, replicated at the 4 row-tile offsets
                pwps = pool_ps.tile([P, 128], f32, name="pwps", tag="Tps",
                                    padded_shape=[128, 128])
                nc.tensor.transpose(pwps[:P, :], t_pw[:], identity[:])
                for q in range(4):
                    nc.vector.tensor_copy(pwt4[g][32 * q:32 * q + P, :],
                                          pwps[:P, :])
                if DEBUG:
                    dma(dbg["dbg_pw"][g * 128:(g + 1) * 128, :], t_pw[:])

            # window scalar -> kk, z, delta
            winw1_sb = pp.tile([128, 4 * 64], f32, name="winw1_sb")
            for kc in range(4):
                dma(winw1_sb[:, kc * 64:(kc + 1) * 64],
                    win_w1[kc * 128:(kc + 1) * 128, :])
            wh1_ps = pool_ps.tile([1, 64], f32, name="wh1ps", tag="Tps",
                                  padded_shape=[128, 128])
            for kc in range(4):
                nc.tensor.matmul(wh1_ps[:1, :], ctxT[:, kc:kc + 1],
                                 winw1_sb[:, kc * 64:(kc + 1) * 64],
                                 start=(kc == 0), stop=(kc == 3))
            wh1 = pp.tile([1, 64], f32, name="wh1")
            wb1_sb = pp.tile([1, 64], f32, name="wb1_sb")
            dma(wb1_sb[:], win_b1[:])
            nc.vector.tensor_tensor(wh1[:], wh1_ps[:1, :], wb1_sb[:], Alu.add)
            gelu_(pp, wh1[:], "wh1g")
            wh1T = pp.tile([64, 1], f32, name="wh1T")
            transpose_to(wh1T[:], wh1[:], "wh1T")
            winw2_sb = pp.tile([64, 1], f32, name="winw2_sb")
            dma(winw2_sb[:], win_w2[:])
            win_ps = pool_ps.tile([1, 1], f32, name="winps", tag="Tps",
                                  padded_shape=[128, 128])
            nc.tensor.matmul(win_ps[:1, :1], wh1T[:], winw2_sb[:], start=True,
                             stop=True)
            winv = pp.tile([1, 1], f32, name="winv")
            wb2_sb = pp.tile([1, 1], f32, name="wb2_sb")
            dma(wb2_sb[:], win_b2[:])
            nc.vector.tensor_tensor(winv[:], win_ps[:1, :1], wb2_sb[:], Alu.add)
            nc.scalar.activation(winv[:], winv[:], Act.Sigmoid)
            nc.vector.tensor_scalar(winv[:], winv[:], float(MAX_SEQ - 256),
                                    256.0, Alu.mult, Alu.add)
            kkf = pp.tile([1, 1], f32, name="kkf")
            nc.vector.tensor_scalar(kkf[:], winv[:], 0.1 / MAX_SEQ * DD, None,
                                    Alu.mult)
            # floor() robust to the f32->i32 convert rounding mode
            ki = pp.tile([1, 1], dt.int32, name="ki")
            nc.vector.tensor_copy(ki[:], kkf[:])
            kf2 = pp.tile([1, 1], f32, name="kf2")
            nc.vector.tensor_copy(kf2[:], ki[:])
            kgt = pp.tile([1, 1], f32, name="kgt")
            nc.vector.tensor_tensor(kgt[:], kf2[:], kkf[:], Alu.is_gt)
            nc.vector.tensor_tensor(kkf[:], kf2[:], kgt[:], Alu.subtract)
            nc.vector.tensor_scalar(kkf[:], kkf[:], 1.0, None, Alu.max)

            qp = pp.tile([1, 4], f32, name="qp")
            dma(qp[:], qpoly[:])
            u = pp.tile([1, 1], f32, name="qu")
            nc.vector.tensor_scalar(u[:], kkf[:], 1.0 / DD, None, Alu.mult)
            nc.scalar.activation(u[:], u[:], Act.Ln)
            zq = pp.tile([1, 1], f32, name="zq")
            nc.vector.tensor_scalar(zq[:], qp[:, 0:1], u[:], qp[:, 1:2],
                                    Alu.mult, Alu.add)
            nc.vector.tensor_scalar(zq[:], zq[:], u[:], qp[:, 2:3], Alu.mult,
                                    Alu.add)
            nc.vector.tensor_scalar(zq[:], zq[:], u[:], qp[:, 3:4], Alu.mult,
                                    Alu.add)
            phi = pp.tile([1, 1], f32, name="phi")
            nc.vector.scalar_tensor_tensor(phi[:], zq[:], -0.5, zq[:],
                                           Alu.mult, Alu.mult)
            nc.scalar.activation(phi[:], phi[:], Act.Exp)
            nc.vector.tensor_scalar(phi[:], phi[:],
                                    float(1.0 / np.sqrt(2 * np.pi)), None,
                                    Alu.mult)
            dens = pp.tile([1, 1], f32, name="dens")
            nc.vector.scalar_tensor_tensor(dens[:], phi[:], float(2.0 * DD),
                                           zq[:], Alu.mult, Alu.mult)
            delta = pp.tile([1, 1], f32, name="delta")
            nc.vector.reciprocal(delta[:], dens[:])
            nc.vector.tensor_scalar(delta[:], delta[:], 700.0, None, Alu.mult)
            pbcast(pp, kk_b[:], kkf[:], 1, "kk")
            pbcast(pp, zq_b[:], zq[:], 1, "zq")
            pbcast(pp, delta_b[:], delta[:], 1, "delta")
            nc.vector.scalar_tensor_tensor(invz2_b[:], zq_b[:], 1.0, zq_b[:],
                                           Alu.mult, Alu.mult)
            nc.vector.reciprocal(invz2_b[:], invz2_b[:])

            if DEBUG:
                for g in range(NG):
                    dma(dbg["dbg_xn"][g * 128:(g + 1) * 128, :], xn[g][:])
                    dma(dbg["dbg_xr"][g * 128:(g + 1) * 128, :], xr[g][:])
                    dma(dbg["dbg_inten"][g * 128:(g + 1) * 128, :], inten[g][:])
                dma(dbg["dbg_scal"][:, 0:1], kkf[:])
                dma(dbg["dbg_scal"][:, 1:2], winv[:])
                dma(dbg["dbg_scal"][:, 2:3], zq[:])
                dma(dbg["dbg_scal"][:, 3:4], delta[:])

        if STAGE < 2:
            for g in range(NG):
                dma(out_dram[g * 128:(g + 1) * 128, :], xg[g][:])
            return nc

        # ===== helper: rematerialize F via 4-way row-tiled K=16 matmuls =====
        def flow_pass(g, consume_pair, wlist=None):
            """consume_pair(c, ps1024) per 1024-span covering chunks c, c+1
            (chunk index = i-row of this core's slice); ps[:, r*512:] holds
            chunk c+r."""
            for w in (wlist if wlist is not None else range(16)):
                for s in range(2):
                    ps = pool_mm.tile([128, 1024], f32, name="Fps", tag="Fps",
                                      padded_shape=[128, 1024])
                    for r in range(2):
                        q = 2 * s + r
                        nc.tensor.matmul(
                            ps[:, r * 512:(r + 1) * 512],
                            pwt4[g][32 * q:32 * q + P, :],
                            patall[32 * q:32 * q + P, w * 512:(w + 1) * 512],
                            start=True, stop=True, tile_position=(32 * q, 0))
                    consume_pair(4 * w + 2 * s, ps)

        t0_stage = pool_dram.tile([S, 1], f32, name="t0_stage")
        t0_out = [pool_dram.tile([128, 1], f32, name=f"t0_out{g}",
                                 addr_space="Shared") for g in range(NG)]
        cnt_stage = pool_dram.tile([S, NL], f32, name="cnt_stage")
        cnt_out = pool_dram.tile([S, NL], f32, name="cnt_out",
                                 addr_space="Shared")
        cand_stage = pool_dram.tile([S, NCAND], f32, name="cand_stage")
        cand_out = pool_dram.tile([NCORES, S, NCAND], f32, name="cand_out",
                                  addr_space="Shared")

        # =============== P1 + selection ladder (scoped pool) ===============
        with tc.tile_pool(name="selpool", bufs=1) as sp:
            A_bf = sp.tile([128, NG * FREE], bf16, name="A_bf")
            vdummy = sp.tile([128, QW], bf16, name="vdummy")
            sdummy = sp.tile([128, QW], bf16, name="sdummy")

            for g in range(NG):
                def consume_p1(c, ps, g=g):
                    nc.scalar.activation(
                        A_bf[:, g * FREE + c * 512: g * FREE + (c + 2) * 512],
                        ps[:], Act.Abs)
                flow_pass(g, consume_p1)
                Ag = A_bf[:, g * FREE:(g + 1) * FREE]
                # sigma estimate from the sample mean: sigma ~ mean|A|*sqrt(pi/2)
                l1s = sp.tile([128, 1], f32, name=f"l1s{g}")
                nc.vector.tensor_reduce(l1s[:], Ag[:, :NEWT_W], AxX, Alu.add)
                t0 = sp.tile([128, 1], f32, name=f"t0{g}")
                nc.vector.tensor_scalar(
                    t0[:], l1s[:], float(np.sqrt(np.pi / 2) / NEWT_W), None,
                    Alu.mult)
                nc.vector.tensor_tensor(t0[:], t0[:], zq_b[:], Alu.mult)

                # Newton on a contiguous sample
                Asmp = Ag[:, :NEWT_W]
                cs = sp.tile([128, 1], f32, name=f"cs{g}")
                lnr = sp.tile([128, 1], f32, name=f"lnr{g}")
                ktgt = sp.tile([128, 1], f32, name=f"ktgt{g}")
                nc.vector.tensor_scalar(
                    ktgt[:], kk_b[:], float(NEWT_W / (NCORES * FREE)), None,
                    Alu.mult)
                rtg = sp.tile([128, 1], f32, name=f"rtg{g}")
                nc.vector.reciprocal(rtg[:], ktgt[:])
                for it in range(4):
                    nc.vector.tensor_scalar(vdummy[:, :NEWT_W], Asmp, t0[:],
                                            None, Alu.is_ge, Alu.add,
                                            accum_out=cs[:])
                    nc.vector.tensor_scalar(cs[:], cs[:], 1.0, None, Alu.max)
                    nc.vector.tensor_tensor(lnr[:], cs[:], rtg[:], Alu.mult)
                    nc.vector.tensor_scalar(lnr[:], lnr[:], 0.1, 10.0, Alu.max,
                                            Alu.min)
                    nc.scalar.activation(lnr[:], lnr[:], Act.Ln)
                    nc.vector.tensor_tensor(lnr[:], lnr[:], invz2_b[:],
                                            Alu.mult)
                    nc.scalar.activation(lnr[:], lnr[:], Act.Exp)
                    nc.vector.tensor_tensor(t0[:], t0[:], lnr[:], Alu.mult)
                dma(t0_stage[g * 128:(g + 1) * 128, :], t0[:])
                # per-group t0 harmonization: g0's all-reduce overlaps g1's P1
                nc.gpsimd.collective_compute(
                    "AllReduce", Alu.add, replica_groups=RG,
                    ins=[t0_stage[g * 128:(g + 1) * 128, :]],
                    outs=[t0_out[g][:]])

            unit = 0  # (g, rung) count-unit index for engine assignment
            for g in range(NG):
                Ag = A_bf[:, g * FREE:(g + 1) * FREE]
                t0 = sp.tile([128, 1], f32, name=f"t0h{g}")
                dma(t0[:], t0_out[g][:])
                nc.vector.tensor_scalar(t0[:], t0[:], 1.0 / NCORES, None,
                                        Alu.mult)
                if DEBUG:
                    dma(dbg["dbg_t0"][g * 128:(g + 1) * 128, :], t0[:])

                # rung ladder on the bf16 grid + f32 count boundaries beta
                tl = sp.tile([128, NL], f32, name=f"tlad{g}")
                tl_bf = sp.tile([128, NL], bf16, name=f"tladbf{g}")
                fac = sp.tile([128, 1], f32, name=f"fac{g}")
                for j in range(NL):
                    nc.vector.tensor_scalar(fac[:], delta_b[:],
                                            float(j - NL // 2), None, Alu.mult)
                    nc.scalar.activation(fac[:], fac[:], Act.Exp)
                    nc.vector.tensor_tensor(tl[:, j:j + 1], t0[:], fac[:],
                                            Alu.mult)
                nc.vector.tensor_copy(tl_bf[:], tl[:])
                nc.vector.tensor_copy(tl[:], tl_bf[:])
                # beta_j = (t + prev16(t))/2: exact f32 count-boundary of the
                # bf16 threshold t (prev16(t) = bf16RTN(t*(1-2^-8)))
                pv = sp.tile([128, NL], f32, name=f"pvl{g}")
                pv_bf = sp.tile([128, NL], bf16, name=f"pvlbf{g}")
                nc.vector.tensor_scalar(pv[:], tl[:], float(1.0 - 2.0 ** -8),
                                        None, Alu.mult)
                nc.vector.tensor_copy(pv_bf[:], pv[:])
                nc.vector.tensor_copy(pv[:], pv_bf[:])
                nc.vector.tensor_tensor(pv[:], pv[:], tl[:], Alu.add)
                nc.vector.tensor_scalar(blad_all[g][:], pv[:], 0.5, None,
                                        Alu.mult)
                nbet = sp.tile([128, NL], f32, name=f"nbet{g}")
                nc.vector.tensor_scalar(nbet[:], blad_all[g][:], -1.0, None,
                                        Alu.mult)

                cl = sp.tile([128, NL], f32, name=f"cl{g}")
                cq = sp.tile([128, NL * NQ], f32, name=f"cq_{g}")
                sq = sp.tile([128, NL * NQ], f32, name=f"sq_{g}")
                for j in range(NL):
                    if unit in SCAL_UNITS:
                        # ACT-engine count: sum of Sign(A - beta_j); exact
                        # since beta_j is strictly between bf16 grid points
                        for qq in range(NQ):
                            nc.scalar.activation(
                                sdummy[:], Ag[:, qq * QW:(qq + 1) * QW],
                                Act.Sign, bias=nbet[:, j:j + 1],
                                accum_out=sq[:, j * NQ + qq: j * NQ + qq + 1])
                        nc.vector.tensor_reduce(
                            cl[:, j:j + 1], sq[:, j * NQ:(j + 1) * NQ], AxX,
                            Alu.add)
                        nc.vector.tensor_scalar(cl[:, j:j + 1], cl[:, j:j + 1],
                                                0.5, float(FREE / 2),
                                                Alu.mult, Alu.add)
                    else:
                        for qq in range(NQ):
                            nc.vector.tensor_scalar(
                                vdummy[:], Ag[:, qq * QW:(qq + 1) * QW],
                                tl[:, j:j + 1], None, Alu.is_ge, Alu.add,
                                accum_out=cq[:, j * NQ + qq: j * NQ + qq + 1])
                        nc.vector.tensor_reduce(
                            cl[:, j:j + 1], cq[:, j * NQ:(j + 1) * NQ], AxX,
                            Alu.add)
                    unit += 1
                dma(cnt_stage[g * 128:(g + 1) * 128, :], cl[:])

        nc.gpsimd.collective_compute(
            "AllReduce", Alu.add, replica_groups=RG,
            ins=[cnt_stage[:]], outs=[cnt_out[:]])

        # bracket selection (small persistent tiles)
        with tc.tile_pool(name="bracket", bufs=1) as bp:
            for g in range(NG):
                cl = bp.tile([128, NL], f32, name=f"clg{g}")
                dma(cl[:], cnt_out[g * 128:(g + 1) * 128, :])
                if DEBUG:
                    dma(dbg["dbg_cnt"][g * 128:(g + 1) * 128, 0:NL], cl[:])
                ge = bp.tile([128, NL], f32, name=f"ge{g}")
                nc.vector.tensor_scalar(ge[:], cl[:], kk_b[:], None, Alu.is_ge)
                sel = bp.tile([128, NL - 1], f32, name=f"sel{g}")
                nc.vector.tensor_scalar(sel[:], ge[:, 1:NL], -1.0, 1.0,
                                        Alu.mult, Alu.add)
                nc.vector.tensor_tensor(sel[:], sel[:], ge[:, 0:NL - 1],
                                        Alu.mult)
                c2 = bp.tile([128, 1], f32, name=f"c2_{g}")
                stmp = bp.tile([128, NL - 1], f32, name=f"stmp{g}")
                bl = blad_all[g]
                nc.vector.tensor_tensor(stmp[:], sel[:], bl[:, 0:NL - 1],
                                        Alu.mult)
                nc.vector.tensor_reduce(beta[g][0][:], stmp[:], AxX, Alu.add)
                nc.vector.tensor_tensor(stmp[:], sel[:], bl[:, 1:NL], Alu.mult)
                nc.vector.tensor_reduce(beta[g][1][:], stmp[:], AxX, Alu.add)
                nc.vector.tensor_tensor(stmp[:], sel[:], cl[:, 1:NL], Alu.mult)
                nc.vector.tensor_reduce(c2[:], stmp[:], AxX, Alu.add)
                nc.vector.scalar_tensor_tensor(rprime[g][:], c2[:], -1.0,
                                               kk_b[:], Alu.mult, Alu.add)
                if DEBUG:
                    dma(dbg["dbg_beta"][g * 128:(g + 1) * 128, 0:1],
                        beta[g][0][:])
                    dma(dbg["dbg_beta"][g * 128:(g + 1) * 128, 1:2],
                        beta[g][1][:])
                    dma(dbg["dbg_beta"][g * 128:(g + 1) * 128, 2:3], c2[:])
                    dma(dbg["dbg_beta"][g * 128:(g + 1) * 128, 3:4],
                        rprime[g][:])

        # ===== P3: band extraction via max8 per 512-chunk (scoped pool) =====
        with tc.tile_pool(name="p3pool", bufs=1) as xp:
            for g in range(NG):
                b2t = beta[g][1]
                cand = xp.tile([128, NCAND], f32, name="cand", tag="cand")
                for h in range(2):
                    A32 = xp.tile([128, HF], f32, name="A32", tag="A32",
                                  bufs=2)

                    def consume_p3(c, ps, h=h, A32=A32):
                        cc = c - h * 32
                        nc.scalar.activation(A32[:, cc * 512:(cc + 2) * 512],
                                             ps[:], Act.Abs)
                    flow_pass(g, consume_p3, wlist=range(8 * h, 8 * h + 8))
                    # in-place band mask: A32 = (A32 < beta2) * A32
                    nc.vector.scalar_tensor_tensor(A32[:], A32[:], b2t[:],
                                                   A32[:], Alu.is_lt, Alu.mult)
                    nch = HF // EXCH
                    for cc in range(nch):
                        nc.vector.max(
                            out=cand[:, (h * nch + cc) * 8:
                                     (h * nch + cc + 1) * 8],
                            in_=A32[:, cc * EXCH:(cc + 1) * EXCH])
                dma(cand_stage[g * 128:(g + 1) * 128, :], cand[:])
                if DEBUG:
                    dma(dbg["dbg_cand"][g * 128:(g + 1) * 128, :], cand[:])

        nc.gpsimd.collective_compute(
            "AllGather", Alu.bypass, replica_groups=RG,
            ins=[cand_stage[:]], outs=[cand_out[:]])

        # =============== exact threshold: replicated bisection ===============
        GW = NCORES * NCAND
        with tc.tile_pool(name="bisect", bufs=1) as gp:
            G, gd = [], []
            for g in range(NG):
                G.append(gp.tile([128, GW], f32, name=f"Gc{g}", tag=f"Gc{g}"))
                gd.append(gp.tile([128, GW], bf16, name=f"gd{g}",
                                  tag=f"gd{g}"))
                for cidx in range(NCORES):
                    dma(G[g][:, cidx * NCAND:(cidx + 1) * NCAND],
                        cand_out[cidx, g * 128:(g + 1) * 128, :])
            # both groups ride in the two lanes of [128,2] state tiles
            lo, hi, mid, nmid, cm, sl, dh, sd, rp2, cHI, sacc = (
                gp.tile([128, 2], f32, name=nm)
                for nm in ("lo2", "hi2", "mid2", "nmid2", "cm2", "sl2", "dh2",
                           "sd2", "rp2", "cHI2", "sacc2"))
            for g in range(NG):
                nc.vector.tensor_copy(lo[:, g:g + 1], beta[g][0][:])
                nc.vector.tensor_copy(hi[:, g:g + 1], beta[g][1][:])
                nc.vector.tensor_copy(rp2[:, g:g + 1], rprime[g][:])

            def step_mid():
                # dh = (hi - lo)/2 ; mid = lo + dh
                nc.vector.tensor_tensor(dh[:], hi[:], lo[:], Alu.subtract)
                nc.vector.tensor_scalar(dh[:], dh[:], 0.5, None, Alu.mult)
                nc.vector.tensor_tensor(mid[:], lo[:], dh[:], Alu.add)

            def upd_lohi():
                # sl = cm>=r' ; sd = sl*dh ; lo += sd ; hi = mid + sd
                nc.vector.tensor_tensor(sl[:], cm[:], rp2[:], Alu.is_ge)
                nc.vector.tensor_tensor(sd[:], sl[:], dh[:], Alu.mult)
                nc.vector.tensor_tensor(lo[:], lo[:], sd[:], Alu.add)
                nc.vector.tensor_tensor(hi[:], mid[:], sd[:], Alu.add)

            for _ in range(N_BISECT):
                step_mid()
                # g0 count on the ACT engine (Sign-sum), g1 on vector —
                # the two big counts run concurrently
                nc.vector.tensor_scalar(nmid[:, 0:1], mid[:, 0:1], -1.0,
                                        None, Alu.mult)
                nc.scalar.activation(gd[0][:], G[0][:], Act.Sign,
                                     bias=nmid[:, 0:1],
                                     accum_out=sacc[:, 0:1])
                nc.vector.tensor_scalar(gd[1][:], G[1][:], mid[:, 1:2],
                                        None, Alu.is_ge, Alu.add,
                                        accum_out=cm[:, 1:2])
                nc.vector.tensor_scalar(cm[:, 0:1], sacc[:, 0:1], 0.5,
                                        float(GW / 2), Alu.mult, Alu.add)
                upd_lohi()

            W8 = [gp.tile([128, 8], f32, name=f"W8{g}") for g in range(NG)]
            w8s = [gp.tile([128, 8], f32, name=f"w8s{g}") for g in range(NG)]
            for g in range(NG):
                # cHI = count(G >= hi)
                nc.vector.tensor_scalar(gd[g][:], G[g][:], hi[:, g:g + 1],
                                        None, Alu.is_ge, Alu.add,
                                        accum_out=cHI[:, g:g + 1])
                # window-mask G to [lo, hi) in place, then top-8
                nc.vector.scalar_tensor_tensor(G[g][:], G[g][:],
                                               lo[:, g:g + 1], G[g][:],
                                               Alu.is_ge, Alu.mult)
                nc.vector.scalar_tensor_tensor(G[g][:], G[g][:],
                                               hi[:, g:g + 1], G[g][:],
                                               Alu.is_lt, Alu.mult)
                nc.vector.max(out=W8[g][:], in_=G[g][:])
            for _ in range(N_BISECT2):
                step_mid()
                for g in range(NG):
                    nc.vector.tensor_scalar(w8s[g][:], W8[g][:],
                                            mid[:, g:g + 1], None, Alu.is_ge,
                                            Alu.add, accum_out=cm[:, g:g + 1])
                nc.vector.tensor_tensor(cm[:], cm[:], cHI[:], Alu.add)
                upd_lohi()
            for g in range(NG):
                nc.vector.tensor_copy(th[g][:], lo[:, g:g + 1])
                if DEBUG:
                    dma(dbg["dbg_th"][g * 128:(g + 1) * 128, 0:1], th[g][:])
                    dma(dbg["dbg_th"][g * 128:(g + 1) * 128, 1:2],
                        rprime[g][:])

        if STAGE < 3:
            for g in range(NG):
                dma(out_dram[g * 128:(g + 1) * 128, :], xg[g][:])
            return nc

        # =============== P4: final masked matvec ===============
        fo_stage = pool_dram.tile([S, ISLICE], f32, name="fo_stage")
        fo_out = pool_dram.tile([NCORES, S, ISLICE], f32, name="fo_out",
                                addr_space="Shared")
        tailP = ctx.enter_context(tc.tile_pool(name="tailP", bufs=1))
        fo_full = [tailP.tile([128, D], f32, name=f"fo_full{g}")
                   for g in range(NG)]

        # --- tail preloads: issued before P4 so the DMAs and the memory-bank
        # mean run under P4 compute instead of serializing after the
        # fo all-gather ---
        n2g_b = bcast_row(tailP, n2_g, D, "n2g_b")
        n2b_b = bcast_row(tailP, n2_b, D, "n2b_b")
        memh_wsb = tailP.tile([128, 4 * D], bf16, name="memh_wsb")
        memc_wsb = tailP.tile([128, 4 * D], bf16, name="memc_wsb")
        memo_wsb = tailP.tile([128, 4 * D], bf16, name="memo_wsb")
        for kc in range(4):
            dma(memh_wsb[:, kc * D:(kc + 1) * D],
                mem_w1b[kc * 128:(kc + 1) * 128, :])
            dma(memc_wsb[:, kc * D:(kc + 1) * D],
                mem_w1cb[kc * 128:(kc + 1) * 128, :])
            dma(memo_wsb[:, kc * D:(kc + 1) * D],
                mem_w2b[kc * 128:(kc + 1) * 128, :])
        memh_bias = bcast_row(tailP, mem_b1, D, "memh_bias")
        memo_bias = bcast_row(tailP, mem_b2, D, "memo_bias")
        memx = tailP.tile([128, 4 * D], f32, name="memx")
        for kc in range(4):
            dma(memx[:, kc * D:(kc + 1) * D],
                memory_bank[kc * 128:(kc + 1) * 128, :])
        mem_ps = pool_ps.tile([1, D], f32, name="memps", tag="Tps",
                              padded_shape=[128, 512])
        for kc in range(4):
            nc.tensor.matmul(mem_ps[:1, :], ones_sb[:],
                             memx[:, kc * D:(kc + 1) * D],
                             start=(kc == 0), stop=(kc == 3))
        memv = tailP.tile([1, D], f32, name="memv")
        nc.vector.tensor_scalar(memv[:], mem_ps[:1, :], 1.0 / 512.0, None,
                                Alu.mult)
        memvT = tailP.tile([128, 4], bf16, name="memvT")
        for kc in range(4):
            transpose_to(memvT[:, kc:kc + 1],
                         memv[:, kc * 128:(kc + 1) * 128], f"memvT{kc}")
        # cvec = memv @ mem_w1[D:2D] (broadcast-context term of the mem MLP)
        cps = pool_ps.tile([1, D], f32, name="cps", tag="Tps",
                           padded_shape=[128, 512])
        for kc in range(4):
            nc.tensor.matmul(cps[:1, :], memvT[:, kc:kc + 1],
                             memc_wsb[:, kc * D:(kc + 1) * D],
                             start=(kc == 0), stop=(kc == 3))
        cvec = tailP.tile([1, D], f32, name="memh_cvec")
        nc.vector.tensor_copy(cvec[:], cps[:1, :])
        memh_cvecb = tailP.tile([128, D], f32, name="memh_cvecb")
        pbcast(tailP, memh_cvecb[:], cvec[:], D, "memhcv")
        # fold the broadcast-context term into the bias once
        nc.vector.tensor_tensor(memh_bias[:], memh_bias[:], memh_cvecb[:],
                                Alu.add)
        with tc.tile_pool(name="p4pool", bufs=1) as fp:
            XI = []
            for g in range(NG):
                t = fp.tile([128, D], f32, name=f"XI{g}")
                nc.vector.tensor_scalar(t[:], xn[g][:], inten[g][:], None,
                                        Alu.mult)
                XI.append(t)
            for g in range(NG):
                FO = fp.tile([128, ISLICE], f32, name=f"FO{g}")

                def consume_p4(c, ps, g=g, FO=FO):
                    At = fp.tile([128, 1024], f32, name="At", tag="At", bufs=3)
                    FM = fp.tile([128, 1024], f32, name="FM", tag="FM", bufs=3)
                    nc.scalar.activation(At[:], ps[:], Act.Abs)
                    nc.vector.scalar_tensor_tensor(FM[:], At[:], th[g][:],
                                                   ps[:], Alu.is_ge, Alu.mult)
                    for r in range(2):
                        nc.vector.scalar_tensor_tensor(
                            FM[:, r * 512:(r + 1) * 512],
                            FM[:, r * 512:(r + 1) * 512], 1.0, XI[g][:],
                            Alu.mult, Alu.mult,
                            accum_out=FO[:, c + r:c + r + 1])
                flow_pass(g, consume_p4)
                dma(fo_stage[g * 128:(g + 1) * 128, :], FO[:])

        nc.gpsimd.collective_compute(
            "AllGather", Alu.bypass, replica_groups=RG,
            ins=[fo_stage[:]], outs=[fo_out[:]])

        # =============== tail ===============
        co = [tailP.tile([128, D], f32, name=f"co{g}") for g in range(NG)]
        with tc.tile_pool(name="tail1", bufs=1) as tp:
            for g in range(NG):
                for cidx in range(NCORES):
                    dma(fo_full[g][:, cidx * ISLICE:(cidx + 1) * ISLICE],
                        fo_out[cidx, g * 128:(g + 1) * 128, :])
                if DEBUG:
                    dma(dbg["dbg_fo"][g * 128:(g + 1) * 128, :], fo_full[g][:])
                nc.vector.tensor_tensor(co[g][:], xg[g][:], fo_full[g][:],
                                        Alu.add)
                mean = tp.tile([128, 1], f32, name=f"mean2{g}")
                m2 = tp.tile([128, 1], f32, name=f"m2ln2{g}")
                tmp = tp.tile([128, D], f32, name=f"ln2tmp{g}", tag="tmp")
                nc.vector.tensor_reduce(mean[:], co[g][:], AxX, Alu.add)
                nc.vector.tensor_scalar(mean[:], mean[:], 1.0 / D, None,
                                        Alu.mult)
                nc.vector.tensor_scalar(tmp[:], co[g][:], mean[:], None,
                                        Alu.subtract)
                nc.vector.scalar_tensor_tensor(tmp[:], tmp[:], 1.0, tmp[:],
                                               Alu.mult, Alu.mult,
                                               accum_out=m2[:])
                nc.vector.tensor_scalar(m2[:], m2[:], 1.0 / D, 1e-5, Alu.mult,
                                        Alu.add)
                rstd = tp.tile([128, 1], f32, name=f"rstd2{g}")
                nc.scalar.activation(rstd[:], m2[:], Act.Sqrt)
                nc.vector.reciprocal(rstd[:], rstd[:])
                nc.vector.tensor_scalar(co[g][:], co[g][:], mean[:], rstd[:],
                                        Alu.subtract, Alu.mult)
                nc.vector.scalar_tensor_tensor(co[g][:], co[g][:], 1.0,
                                               n2g_b[:], Alu.mult, Alu.mult)
                nc.vector.tensor_tensor(co[g][:], co[g][:], n2b_b[:], Alu.add)

        def transposed_cols(pool, src_list, K, name):
            nk = K // 128
            tT = pool.tile([128, nk * S], bf16, name=f"{name}_T")
            for g in range(NG):
                for kc in range(nk):
                    transpose_to(tT[:, kc * S + g * 128: kc * S + (g + 1) * 128],
                                 src_list[g][:, kc * 128:(kc + 1) * 128],
                                 f"{name}T{g}_{kc}")
            return lambda g, kc: tT[:, kc * S + g * 128: kc * S + (g + 1) * 128]

        def big_matmul(pool, lhsT_cols, w_dram, K, N, name, bias_dram=None,
                       out_list=None, wsb_pre=None, bias_pre=None,
                       w_col0=0, wsb_tag=None, bias_tag=None, out_tag=None):
            nk = K // 128
            if wsb_pre is not None:
                wsb = wsb_pre
            else:
                wsb = pool.tile([128, nk * N], bf16, name=f"{name}_wsb",
                                tag=wsb_tag)
                for kc in range(nk):
                    dma(wsb[:, kc * N:(kc + 1) * N],
                        w_dram[kc * 128:(kc + 1) * 128,
                               w_col0:w_col0 + N])
            bias_b = bias_pre
            if bias_b is None and bias_dram is not None:
                bias_b = bcast_row(pool, bias_dram, N, f"{name}_bias",
                                   tag=bias_tag)
            outs = []
            for g in range(NG):
                o = (out_list[g] if out_list is not None
                     else pool.tile([128, N], f32, name=f"{name}_o{g}",
                                    tag=out_tag,
                                    bufs=2 if out_tag else 1))
                for nb in range(0, N, 512):
                    nw = min(512, N - nb)
                    ps = pool_mm.tile([128, nw], f32, name="Fps", tag="Fps",
                                      padded_shape=[128, 1024])
                    for kc in range(nk):
                        nc.tensor.matmul(ps[:], lhsT_cols(g, kc),
                                         wsb[:, kc * N + nb: kc * N + nb + nw],
                                         start=(kc == 0), stop=(kc == nk - 1))
                    if bias_b is not None:
                        # fused psum copy + bias add
                        nc.vector.scalar_tensor_tensor(
                            o[:, nb:nb + nw], ps[:], 1.0,
                            bias_b[:, nb:nb + nw], Alu.mult, Alu.add)
                    else:
                        nc.vector.tensor_copy(o[:, nb:nb + nw], ps[:])
                outs.append(o)
            return outs

        with tc.tile_pool(name="tailA", bufs=1) as ta_:
            coT = transposed_cols(ta_, co, D, "coT")
            mh = big_matmul(ta_, coT, mem_w1, D, D, "memh",
                            wsb_pre=memh_wsb, bias_pre=memh_bias)
            for g in range(NG):
                silu_(ta_, mh[g][:], mh[g][:], f"mh{g}")
            mhT = transposed_cols(ta_, mh, D, "mhT")
            mo = big_matmul(ta_, mhT, mem_w2, D, D, "memo",
                            wsb_pre=memo_wsb, bias_pre=memo_bias)
            for g in range(NG):
                nc.vector.tensor_tensor(co[g][:], co[g][:], mo[g][:], Alu.add)

        gv = [tailP.tile([128, 4 * D], f32, name=f"gv{g}") for g in range(NG)]
        with tc.tile_pool(name="tailB", bufs=1) as tb_:
            coT2 = transposed_cols(tb_, co, D, "coT2")
            # up-proj split into gate / val halves to halve SBUF pressure
            gate_b = bcast_row(tb_, up_b, 4 * D, "gate_bias", tag="ffbias")
            ff_g = big_matmul(tb_, coT2, up_wb, D, 4 * D, "ffg",
                              bias_pre=gate_b, wsb_tag="ffwsb",
                              out_tag="ffo")
            for g in range(NG):
                silu_(tb_, gv[g][:], ff_g[g][:], f"gv{g}")
            val_b = bcast_row(tb_, up_b, 4 * D, "val_bias", tag="ffbias",
                              col0=4 * D)
            ff_v = big_matmul(tb_, coT2, up_wb, D, 4 * D, "ffv",
                              bias_pre=val_b, wsb_tag="ffwsb",
                              out_tag="ffo", w_col0=4 * D)
            for g in range(NG):
                nc.vector.tensor_tensor(gv[g][:], gv[g][:], ff_v[g][:],
                                        Alu.mult)
        with tc.tile_pool(name="tailC", bufs=1) as tcp:
            gvT = transposed_cols(tcp, gv, 4 * D, "gvT")
            ffn = big_matmul(tcp, gvT, down_wb, 4 * D, D, "ffn",
                             bias_dram=down_b)
            for g in range(NG):
                nc.vector.tensor_tensor(ffn[g][:], ffn[g][:], co[g][:],
                                        Alu.add)
                dma(out_dram[g * 128:(g + 1) * 128, :], ffn[g][:])

    return nc


def _install_ntff_shim():
    """Reconstitute the missing antenv.axon_hooks module so
    run_bass_kernel_spmd(trace=True) can reach the axon NTFF profiler."""
    import sys
    import types

    if "antenv.axon_hooks" in sys.modules:
        return
    import antenv

    mod = types.ModuleType("antenv.axon_hooks")
    _h = [None]
    mod.set_axon_ntff_profile_hook = lambda h: _h.__setitem__(0, h)
    mod.get_axon_ntff_profile_hook = lambda: _h[0]
    sys.modules["antenv.axon_hooks"] = mod
    antenv.axon_hooks = mod
    try:
        from trn_agent_boot.trn_boot import _ntff_profile_via_ctypes

        mod.set_axon_ntff_profile_hook(
            _ntff_profile_via_ctypes("/opt/axon/libaxon_pjrt.so"))
    except Exception:
        pass


def kernel(**inputs):
    from concourse.bass_utils import run_bass_kernel_spmd
    _install_ntff_shim()

    sin, cos, qpoly = _host_constants()
    x = np.ascontiguousarray(np.asarray(inputs["x"], np.float32).reshape(S, D))
    patterns = np.ascontiguousarray(np.asarray(inputs["flow_patterns"], np.float32))

    nc = build_kernel()
    nc.finalize()

    def a(k):
        return np.ascontiguousarray(np.asarray(inputs[k], np.float32))

    def row(k):
        return np.ascontiguousarray(np.asarray(inputs[k], np.float32).reshape(1, -1))

    base = {
        "x": x,
        "sel_w1": a("sel_w1"), "sel_b1": row("sel_b1"),
        "sel_w2": a("sel_w2"), "sel_b2": row("sel_b2"),
        "win_w1": a("win_w1"), "win_b1": row("win_b1"),
        "win_w2": a("win_w2"), "win_b2": row("win_b2"),
        "int_w1": a("int_w1"), "int_b1": row("int_b1"),
        "int_w2": a("int_w2"), "int_b2": row("int_b2"),
        "mem_w1": a("mem_w1"), "mem_b1": row("mem_b1"),
        "mem_w2": a("mem_w2"), "mem_b2": row("mem_b2"),
        "memory_bank": a("memory_bank"),
        "up_w": a("up_w"), "up_b": row("up_b"),
        "down_w": a("down_w"), "down_b": row("down_b"),
        "n1_g": row("n1_g"), "n1_b": row("n1_b"),
        "n2_g": row("n2_g"), "n2_b": row("n2_b"),
        "rope_sin": sin, "rope_cos": cos,
        "qpoly": qpoly.reshape(1, 4),
    }

    def to_bf16(v):
        import ml_dtypes
        return np.ascontiguousarray(
            np.asarray(v, np.float32).astype(ml_dtypes.bfloat16))

    mw1 = a("mem_w1")
    base["mem_w1b"] = to_bf16(mw1[:D, :])
    base["mem_w1cb"] = to_bf16(mw1[D:, :])
    base["mem_w2b"] = to_bf16(a("mem_w2"))
    base["up_wb"] = to_bf16(a("up_w"))
    base["down_wb"] = to_bf16(a("down_w"))
    in_maps = []
    for c in range(NCORES):
        m = dict(base)
        m["pat_sl"] = np.ascontiguousarray(
            patterns[:, c * ISLICE:(c + 1) * ISLICE, :].reshape(P, FREE))
        in_maps.append(m)

    trace = os.environ.get("KERNEL_TRACE", "0") == "1"
    res = run_bass_kernel_spmd(nc, in_maps, list(range(NCORES)), trace=trace)
    out0 = res.results[0]
    kernel.last_results = res.results
    kernel.last_exec_ns = getattr(res, "exec_time_ns", None)
    return out0["out"].reshape(B, S, D).astype(np.float32)


if __name__ == "__main__":
    data = np.load("/tmp/inputs.npz")
    inputs = {k: data[k] for k in data.files}
    out = kernel(**inputs)
    print("out", out.shape, float(np.abs(out).max()))
